# revision 1
# baseline (speedup 1.0000x reference)
"""Trainium2 Bass kernel for nn_DRA_52905407152670.

3-layer aspect-attention GRU stack over (B,S,H)=(64,512,768). Data-parallel
over batch across 8 NeuronCores (8 batches/core), weights replicated (f16).
Big tensors are fp16 on-chip with fp32 PSUM accumulation; per-batch vectors
(c, A, softmax stats, h) stay fp32. Self-contained; includes the walrus
sync-wait-limit workaround.
"""
import json as _json
import sys as _sys

_sys.path.insert(0, '/opt/trn_rl_repo')

from concourse import tile as _tile_mod
from concourse import mybir as _mybir
from concourse.tile import ScopedClock as _ScopedClock

_MAX_WAITS = 1
_ws_counter = [0]


def _patched_drain_and_barrier(self, tick_clock, wait_clock):
    nc = self.nc
    carrier = nc.sync.nop(nofuse=True, hint="drain_wait_carrier")
    wait_clock.add_sem_waits(carrier.ins,
                             _ScopedClock({None: tick_clock.global_clock}))
    si = carrier.ins.sync_info
    waits = list(si.on_wait) if si is not None else []
    if len(waits) > _MAX_WAITS:
        carrier.ins.sync_info = _mybir.SyncInfo(
            on_wait=waits[:_MAX_WAITS], on_update=list(si.on_update))
        rest = waits[_MAX_WAITS:]
        for i in range(0, len(rest), _MAX_WAITS):
            extra = nc.sync.nop(nofuse=True, hint=f"drain_wait_{i}")
            extra.ins.sync_info = _mybir.SyncInfo(
                on_wait=rest[i:i + _MAX_WAITS], on_update=[])
    nc.sync.drain()
    nc.all_engine_barrier()
    assert self.sems is not None
    popped = nc._tile_sem_poison_stack.pop()
    assert popped is self._sem_poison
    nc.clear_and_free_semaphores(list(self.sems.allocated().values()))
    nc.all_engine_barrier()


_tile_mod.TileContext._drain_and_barrier = _patched_drain_and_barrier


def _split_bir_waits(bir_str):
    d = _json.loads(bir_str)
    changed = False
    for fn in d.get('functions', []):
        for blk in fn.get('blocks', []):
            out = []
            for inst in blk.get('instructions', []):
                si = inst.get('sync_info') or {}
                waits = si.get('on_wait') or []
                if len(waits) > _MAX_WAITS:
                    changed = True
                    excess, keep = waits[:-_MAX_WAITS], waits[-_MAX_WAITS:]
                    for i in range(0, len(excess), _MAX_WAITS):
                        _ws_counter[0] += 1
                        out.append({
                            "debug": inst.get("debug", 0),
                            "engine": inst["engine"],
                            "ins": [], "outs": [],
                            "name": f"I-wsplit{_ws_counter[0]}",
                            "opcode": "NoOp",
                            "sync_info": {"on_update": [],
                                          "on_wait": excess[i:i + _MAX_WAITS]},
                            "text_hint": "wait_split",
                        })
                    si = dict(si)
                    si['on_wait'] = keep
                    inst = dict(inst)
                    inst['sync_info'] = si
                out.append(inst)
            blk['instructions'] = out
    return _json.dumps(d) if changed else bir_str


import concourse.bass2jax as _b2j
import concourse.bass_utils as _bu

_orig_compile = _bu.compile_bir_kernel


def _patched_compile(bir_str, *a, **k):
    was_bytes = isinstance(bir_str, (bytes, bytearray))
    out = _split_bir_waits(bir_str.decode() if was_bytes else bir_str)
    return _orig_compile(out.encode() if was_bytes else out, *a, **k)


if getattr(_bu.compile_bir_kernel, '__name__', '') != '_patched_compile':
    _bu.compile_bir_kernel = _patched_compile
    _b2j.compile_bir_kernel = _patched_compile



import math
import sys

sys.path.insert(0, '/opt/trn_rl_repo')

import numpy as np
import concourse.bass as bass
import concourse.mybir as mybir
from concourse import tile
from concourse.masks import make_identity

dt = mybir.dt
AF = mybir.ActivationFunctionType
ALU = mybir.AluOpType
AX = mybir.AxisListType
P = 128


def chunks(total, maxc=512):
    out = []
    c0 = 0
    while c0 < total:
        cl = min(maxc, total - c0)
        out.append((c0, cl))
        c0 += cl
    return out


def build_nc(NB, S, H, G, LAYERS, NCORES=8):
    KS, SB, GS = H // P, S // P, (3 * G)
    NGRP = (NB + 3) // 4
    nc = bass.Bass("TRN2", target_bir_lowering=False, debug=False,
                   num_devices=NCORES)

    ap_x = nc.declare_dram_parameter("x", [NB, S, H], dt.float32, isOutput=False)
    ap_sr = nc.declare_dram_parameter("sr", [NB, H], dt.float32, isOutput=False)
    ap_asp = nc.declare_dram_parameter("asp", [NB, H], dt.float32, isOutput=False)
    ap_mask = nc.declare_dram_parameter("mask", [NB, S], dt.float32, isOutput=False)
    ap_ws = nc.declare_dram_parameter("ws", [H, H], dt.float16, isOutput=False)
    ap_wa = nc.declare_dram_parameter("wa", [H, H], dt.float16, isOutput=False)
    ap_wd1 = nc.declare_dram_parameter("wd1", [H, H], dt.float16, isOutput=False)
    ap_wd = nc.declare_dram_parameter("wd", [H, G], dt.float16, isOutput=False)
    ap_whs = nc.declare_dram_parameter("whs", [H, G], dt.float16, isOutput=False)
    ap_wihT = nc.declare_dram_parameter("wihT", [H, GS], dt.float16, isOutput=False)
    ap_whhT = nc.declare_dram_parameter("whhT", [G, GS], dt.float16, isOutput=False)
    ap_w = nc.declare_dram_parameter("w", [H], dt.float32, isOutput=False)
    ap_out = nc.declare_dram_parameter("out", [NB, G], dt.float32, isOutput=True)

    with tile.TileContext(nc) as tc:
        _emit(tc, nc, locals(), NB, S, H, G, LAYERS)
    return nc


def _emit(tc, nc, aps, NB, S, H, G, LAYERS):
    KS, SB, GSL = H // P, S // P, G // P
    G3 = 3 * G
    NGRP = (NB + 3) // 4
    f16, f32 = dt.float16, dt.float32

    def grp_members(g):
        return list(range(4 * g, min(4 * g + 4, NB)))

    from contextlib import ExitStack
    ctx = ExitStack()

    # ---------------- resident pool ----------------
    res = ctx.enter_context(tc.tile_pool(name="res", bufs=1))

    ident16 = res.tile([P, P], f16, tag="id16", name="ident16")
    make_identity(nc, ident16)
    ident32 = res.tile([P, P], f32, tag="id32", name="ident32")
    make_identity(nc, ident32)

    wvec = res.tile([P, KS], f16, tag="wvec", name="wvec")

    # mask tiles: rows of group g at partitions {0,32,64,96}
    maskt, dinv = [], []
    for g in range(NGRP):
        mt = res.tile([P, S], f16, tag=f"maskt{g}", name=f"maskt{g}")
        nc.gpsimd.memset(mt[:, :], 0.0)
        for j, b in enumerate(grp_members(g)):
            nc.gpsimd.dma_start(out=mt[32 * j:32 * j + 1, :],
                                in_=aps['ap_mask'][b:b + 1, :])
        maskt.append(mt)
        den = res.tile([P, 1], f32, tag=f"den{g}", name=f"den{g}")
        nc.vector.tensor_reduce(out=den, in_=mt[:, :],
                                axis=AX.X, op=ALU.add)
        dv = res.tile([P, 1], f32, tag=f"dinv{g}", name=f"dinv{g}")
        nc.vector.reciprocal(out=dv, in_=den)
        dinv.append(dv)

    srTf = [res.tile([P, NB], f16, tag=f"srT{hs}", name=f"srT{hs}")
            for hs in range(KS)]
    aspTf = [res.tile([P, NB], f16, tag=f"aspT{hs}", name=f"aspT{hs}")
             for hs in range(KS)]

    # resident big tensors
    xbf = [res.tile([P, SB * H], f16, tag=f"xbf{b}", name=f"xbf{b}") for b in range(NB)]
    se = [[res.tile([P, S], f16, tag=f"se{b}_{ks}", name=f"se{b}_{ks}")
           for ks in range(KS)] for b in range(NB)]

    AT = [res.tile([P, NB], f32, tag=f"AT{ks}", name=f"AT{ks}") for ks in range(KS)]
    hT = [res.tile([P, NB], f16, tag=f"hT{ks}", name=f"hT{ks}") for ks in range(KS)]
    h_sb = res.tile([NB, G], f32, tag="h_sb", name="h_sb")

    # -------- GRU/Wd weight pool: created BEFORE phase-A pool so its SBUF
    # zone is disjoint (no released-zone dep) and its DMAs can run during
    # Se compute. DMAs are emitted after the x loads (queue order).
    gruP = ctx.enter_context(tc.tile_pool(name="gruP", bufs=1))
    wihTf = [gruP.tile([P, G3], f16, tag=f"wih{hs}", name=f"wihTf{hs}") for hs in range(KS)]
    whhTf = [gruP.tile([P, G3], f16, tag=f"whh{hs}", name=f"whhTf{hs}") for hs in range(GSL)]
    wdf = [[gruP.tile([P, H], f16, tag=f"wd{t}_{hs}", name=f"wdf{t}_{hs}")
            for hs in range(KS)] for t in range(2)]

    # ---------------- phase A ----------------
    with tc.tile_pool(name="phA", bufs=1) as pA, \
         tc.tile_pool(name="psA", bufs=1, space="PSUM") as psA:
        # small inputs: fat-descriptor row loads, cast f16, PE-transpose.
        # Staged via pA/psA tags so no pool-release deps gate phase A.
        wrow32 = pA.tile([1, H], f32, tag="st32", bufs=2, name="wrow32")
        nc.sync.dma_start(out=wrow32[:, :],
                          in_=aps['ap_w'][:].rearrange("(o a) -> o a", o=1))
        wrow = pA.tile([1, H], f16, tag="stf16", bufs=2, name="wrow")
        nc.vector.tensor_copy(wrow[:, :], wrow32[:, :])
        for ks in range(KS):
            tpw = psA.tile([P, 1], f16, tag="smallA", bufs=KS,
                           name=f"tpw{ks}_{nc.next_id()}")
            nc.tensor.transpose(tpw[:, :], wrow[:, ks * P:(ks + 1) * P],
                                ident16[0:1, 0:1])
            nc.vector.tensor_copy(wvec[:, ks:ks + 1], tpw[:, :])
        sr32 = pA.tile([NB, H], f32, tag="st32", bufs=2, name="sr32")
        nc.sync.dma_start(out=sr32[:, :], in_=aps['ap_sr'][:, :])
        srf = pA.tile([NB, H], f16, tag="stf16", bufs=2, name="srf")
        nc.vector.tensor_copy(srf[:, :], sr32[:, :])
        for hs in range(KS):
            tsr = psA.tile([P, NB], f16, tag="smallA", bufs=KS,
                           name=f"tsr{hs}_{nc.next_id()}")
            nc.tensor.transpose(tsr[:, :], srf[:, hs * P:(hs + 1) * P],
                                ident16[0:NB, 0:NB])
            nc.vector.tensor_copy(srTf[hs][:, :], tsr[:, :])
        asp32 = pA.tile([NB, H], f32, tag="st32", bufs=2, name="asp32")
        nc.sync.dma_start(out=asp32[:, :], in_=aps['ap_asp'][:, :])
        aspf = pA.tile([NB, H], f16, tag="stf16", bufs=2, name="aspf")
        nc.vector.tensor_copy(aspf[:, :], asp32[:, :])
        for hs in range(KS):
            tas = psA.tile([P, NB], f16, tag="smallA", bufs=KS,
                           name=f"tas{hs}_{nc.next_id()}")
            nc.tensor.transpose(tas[:, :], aspf[:, hs * P:(hs + 1) * P],
                                ident16[0:NB, 0:NB])
            nc.vector.tensor_copy(aspTf[hs][:, :], tas[:, :])
        # x loads first on the SWDGE queue (cast f32->f16 in the DMA), split
        # per s-block so each batch's chunks land on parallel queues and
        # early batches complete first.
        for b in range(NB):
            for sb in range(SB):
                nc.gpsimd.dma_start(
                    out=xbf[b][:, sb * H:(sb + 1) * H],
                    in_=aps['ap_x'][b][sb * P:(sb + 1) * P, :])

        # ws: direct f16 loads (weights are f16 in DRAM)
        wsf = []
        for hs in range(KS):
            t1 = pA.tile([P, H], f16, tag=f"ws{hs}", name=f"wsf{hs}")
            nc.sync.dma_start(out=t1[:, :], in_=aps['ap_ws'][hs * P:(hs + 1) * P, :])
            wsf.append(t1)

        # A^T from streamed f16 wa chunks
        psa = [psA.tile([P, NB], f32, tag="smallA", bufs=KS, name=f"psa{ks}")
               for ks in range(KS)]
        for hs in range(KS):
            wa16 = pA.tile([P, H], f16, tag="wa16", bufs=1, name=f"wa16_{hs}")
            nc.sync.dma_start(out=wa16[:, :], in_=aps['ap_wa'][hs * P:(hs + 1) * P, :])
            for ks in range(KS):
                nc.tensor.matmul(psa[ks][:, :],
                                 lhsT=wa16[:, ks * P:(ks + 1) * P],
                                 rhs=aspTf[hs][:, :],
                                 start=(hs == 0), stop=(hs == KS - 1))
        for ks in range(KS):
            nc.vector.tensor_copy(AT[ks][:, :], psa[ks][:, :])

        # wd on sync-HWDGE; big GRU mats on scalar-HWDGE (parallel queue)
        for t, apn in enumerate(['ap_wd1', 'ap_wd']):
            for hs in range(KS):
                nc.sync.dma_start(out=wdf[t][hs][:, :],
                                  in_=aps[apn][hs * P:(hs + 1) * P, :])
        for hs in range(KS):
            nc.scalar.dma_start(out=wihTf[hs][:, :],
                                in_=aps['ap_wihT'][hs * P:(hs + 1) * P, :])
        for hs in range(GSL):
            nc.scalar.dma_start(out=whhTf[hs][:, :],
                                in_=aps['ap_whhT'][hs * P:(hs + 1) * P, :])

        # x transpose + Se (pairs of b, xT double-buffered)
        for g4 in range((NB + 1) // 2):
            members = list(range(2 * g4, min(2 * g4 + 2, NB)))
            xTt = {}
            for j, b in enumerate(members):
                xt = pA.tile([P, SB * KS, P], f16, tag="xT", bufs=2,
                             name=f"xT{g4}_{j}")
                nc.sync.dma_start_transpose(out=xt[:, :, :], in_=xbf[b][:, :])
                xTt[b] = xt
            for ks in range(KS):
                for b in members:
                    pse = psA.tile([P, S], f32, tag="seps", bufs=2, name=f"pse{b}_{ks}")
                    for hs in range(KS):
                        nc.tensor.matmul(pse[:, :],
                                         lhsT=wsf[hs][:, ks * P:(ks + 1) * P],
                                         rhs=xTt[b][:, hs::KS, :],
                                         start=(hs == 0), stop=(hs == KS - 1))
                    nc.vector.tensor_copy(se[b][ks][:, :], pse[:, :])

    lay = ctx.enter_context(tc.tile_pool(name="lay", bufs=1))
    psL = ctx.enter_context(tc.tile_pool(name="psL", bufs=1, space="PSUM"))

    for t in range(LAYERS):
        wdW = wdf[min(t, 1)]
        hT_in = srTf if t == 0 else hT

        # cT[ks][:, b] = (h @ Wd + A)^T column
        cT = []
        for ks in range(KS):
            psc = psL.tile([P, NB], f32, tag="small", bufs=2, name=f"psc{t}_{ks}")
            for hs in range(KS):
                nc.tensor.matmul(psc[:, :], lhsT=wdW[hs][:, ks * P:(ks + 1) * P],
                                 rhs=hT_in[hs][:, :],
                                 start=(hs == 0), stop=(hs == KS - 1))
            ct = lay.tile([P, NB], f32, tag=f"cT{ks}", bufs=2, name=f"cT{t}_{ks}")
            nc.vector.tensor_add(ct[:, :], psc[:, :], AT[ks][:, :])
            cT.append(ct)

        for g in range(NGRP):
            members = grp_members(g)
            # tanh + matvec into col groups of one PSUM tile
            scps = psL.tile([P, S], f32, tag="sc", bufs=2, name=f"scps{t}_{g}")
            for j, b in enumerate(members):
                for ks in range(KS):
                    th = lay.tile([P, S], f16, tag="th", bufs=5, name=f"th{t}_{b}_{ks}")
                    nc.scalar.activation(th[:, :], se[b][ks][:, :], AF.Tanh,
                                         bias=cT[ks][:, b:b + 1], scale=1.0)
                    nc.tensor.matmul(scps[32 * j:32 * j + 1, :],
                                     lhsT=wvec[:, ks:ks + 1], rhs=th[:, :],
                                     start=(ks == 0), stop=(ks == KS - 1),
                                     tile_position=(0, 32 * j))
            # softmax (rows {32j}); garbage rows harmless
            negmax = lay.tile([P, 1], f32, tag="negmax", bufs=2, name=f"negmax{t}_{g}")
            nc.vector.tensor_reduce(out=negmax, in_=scps[:, :], axis=AX.X,
                                    op=ALU.max, negate=True)
            m = lay.tile([P, S], f16, tag="m", bufs=2, name=f"m{t}_{g}")
            nc.scalar.activation(m[:, :], scps[:, :], AF.Exp, bias=negmax, scale=1.0)
            ssum = lay.tile([P, 1], f32, tag="ssum", bufs=2, name=f"ssum{t}_{g}")
            nc.vector.tensor_reduce(out=ssum, in_=m[:, :], axis=AX.X, op=ALU.add)
            sinv = lay.tile([P, 1], f32, tag="sinv", bufs=2, name=f"sinv{t}_{g}")
            nc.vector.reciprocal(out=sinv, in_=ssum)
            scl = lay.tile([P, 1], f32, tag="scl", bufs=2, name=f"scl{t}_{g}")
            nc.vector.tensor_mul(scl[:, :], sinv[:, :], dinv[g][:, :])
            mm = lay.tile([P, S], f16, tag="mm", bufs=2, name=f"mm{t}_{g}")
            nc.vector.tensor_mul(mm[:, :], m[:, :], maskt[g][:, :])
            mwf = lay.tile([P, S], f16, tag="mwf", bufs=2, name=f"mwf{t}_{g}")
            nc.vector.tensor_scalar_mul(mwf[:, :], mm[:, :], scl[:, :])

            # mwT: transpose each 128-s block (f16 PE transpose)
            mwT = []
            for sb in range(SB):
                tps = psL.tile([P, P], f16, tag="small", bufs=2, name=f"tps{t}_{g}_{sb}")
                nc.tensor.transpose(tps[:, :], mwf[:, sb * P:(sb + 1) * P], ident16[:, :])
                mt = lay.tile([P, P], f16, tag=f"mwT{sb}", bufs=2, name=f"mwT{t}_{g}_{sb}")
                nc.vector.tensor_copy(mt[:, :], tps[:, :])
                mwT.append(mt)

            # at: for each b, contract over s
            atps = psL.tile([P, H], f32, tag="at", bufs=1, name=f"atps{t}_{g}")
            for j, b in enumerate(members):
                for c0, cl in chunks(H):
                    for sb in range(SB):
                        nc.tensor.matmul(atps[32 * j:32 * j + 1, c0:c0 + cl],
                                         lhsT=mwT[sb][:, 32 * j:32 * j + 1],
                                         rhs=xbf[b][:, sb * H + c0: sb * H + c0 + cl],
                                         start=(sb == 0), stop=(sb == SB - 1),
                                         tile_position=(0, 32 * j))
            asb = lay.tile([P, H], f32, tag="asb", bufs=1, name=f"asb{t}_{g}")
            nc.vector.tensor_copy(asb[:, :], atps[:, :])
            # atT -> dense f16 (P, NB) tiles
            if g == 0:
                atTd = [lay.tile([P, NB], f16, tag=f"atTd{ks}", bufs=2,
                                 name=f"atTd{t}_{ks}") for ks in range(KS)]
            for ks in range(KS):
                tpa = psL.tile([P, P], f32, tag="small", bufs=2, name=f"tpa{t}_{g}_{ks}")
                nc.tensor.transpose(tpa[:, :], asb[:, ks * P:(ks + 1) * P], ident32[:, :])
                nc.vector.tensor_copy(atTd[ks][:, 4 * g:4 * g + len(members)],
                                      tpa[:, 0:32 * len(members):32])

        # ---- h0 = h @ whs (layer 0 only, before GRU) ----
        if t == 0:
            with tc.tile_pool(name="whsP", bufs=1) as whsP:
                h0ps = psL.tile([NB, G], f32, tag="at", bufs=1, name="h0ps")
                for hs in range(KS):
                    whst16 = whsP.tile([P, G], f16, tag="whs16", bufs=2,
                                       name=f"whst16_{hs}")
                    nc.sync.dma_start(out=whst16[:, :],
                                      in_=aps['ap_whs'][hs * P:(hs + 1) * P, :])
                    for c0, cl in chunks(G):
                        nc.tensor.matmul(h0ps[:, c0:c0 + cl],
                                         lhsT=srTf[hs][:, :],
                                         rhs=whst16[:, c0:c0 + cl],
                                         start=(hs == 0), stop=(hs == KS - 1))
                nc.vector.tensor_copy(h_sb[:, :], h0ps[:, :])
                _update_hT(tc, nc, lay, psL, h_sb, hT, ident32, NB, G)

        # ---- GRU ----
        # r, z: merged gi+gh accumulation; sigmoid straight from PSUM
        # r and z gates are both sigmoid: process [0, 2G) as one block
        rz = lay.tile([NB, 2 * G], f16, tag="g_rz", bufs=1, name=f"grz{t}")
        for c0, cl in chunks(2 * G):
            ps = psL.tile([NB, 512], f32, tag="b1", bufs=2, name=f"psrz{t}_{c0}")
            for hs in range(KS):
                nc.tensor.matmul(ps[:, :cl], lhsT=atTd[hs][:, :],
                                 rhs=wihTf[hs][:, c0:c0 + cl],
                                 start=(hs == 0), stop=False)
            for hs in range(GSL):
                nc.tensor.matmul(ps[:, :cl], lhsT=hT[hs][:, :],
                                 rhs=whhTf[hs][:, c0:c0 + cl],
                                 start=False, stop=(hs == GSL - 1))
            nc.scalar.activation(rz[:, c0:c0 + cl], ps[:, :cl], AF.Sigmoid)
        gate_sb = {'r': rz[:, 0:G], 'z': rz[:, G:2 * G]}
        # n: tanh(gi_n + r * gh_n)
        n_sb = lay.tile([NB, G], f16, tag="g_n", bufs=1, name=f"gn{t}")
        for c0, cl in chunks(G):
            psgi = psL.tile([NB, 512], f32, tag="b1", bufs=2, name=f"psgi{t}_{c0}")
            for hs in range(KS):
                nc.tensor.matmul(psgi[:, :cl], lhsT=atTd[hs][:, :],
                                 rhs=wihTf[hs][:, 2 * G + c0: 2 * G + c0 + cl],
                                 start=(hs == 0), stop=(hs == KS - 1))
            psgh = psL.tile([NB, 512], f32, tag="b1", bufs=2, name=f"psgh{t}_{c0}")
            for hs in range(GSL):
                nc.tensor.matmul(psgh[:, :cl], lhsT=hT[hs][:, :],
                                 rhs=whhTf[hs][:, 2 * G + c0: 2 * G + c0 + cl],
                                 start=(hs == 0), stop=(hs == GSL - 1))
            tmp = lay.tile([NB, 512], f32, tag="gtmp", bufs=2, name=f"gtmp{t}_{c0}")
            nc.vector.tensor_mul(tmp[:, :cl], gate_sb['r'][:, c0:c0 + cl], psgh[:, :cl])
            nc.vector.tensor_add(tmp[:, :cl], tmp[:, :cl], psgi[:, :cl])
            nc.scalar.activation(n_sb[:, c0:c0 + cl], tmp[:, :cl], AF.Tanh)
        # h' = n + z * (h - n)
        hmn = lay.tile([NB, G], f32, tag="hmn", bufs=1, name=f"hmn{t}")
        nc.vector.tensor_sub(hmn[:, :], h_sb[:, :], n_sb[:, :])
        nc.vector.tensor_mul(hmn[:, :], gate_sb['z'], hmn[:, :])
        nc.vector.tensor_add(h_sb[:, :], n_sb[:, :], hmn[:, :])
        if t < LAYERS - 1:
            _update_hT(tc, nc, lay, psL, h_sb, hT, ident32, NB, G)

    nc.sync.dma_start(out=aps['ap_out'][:, :], in_=h_sb[:, :])
    ctx.close()


def _update_hT(tc, nc, lay, psL, h_sb, hT, ident32, NB, G):
    for hs in range(G // P):
        tph = psL.tile([P, NB], dt.float32, tag="small", bufs=2,
                       name=f"tph{hs}_{nc.next_id()}")
        nc.tensor.transpose(tph[:, :], h_sb[:, hs * P:(hs + 1) * P],
                            ident32[0:NB, 0:NB])
        nc.vector.tensor_copy(hT[hs][:, :], tph[:, :])


# ---------------- host side ----------------

def make_in_maps(inputs, NB, S, H, G, NCORES=8):
    x = np.ascontiguousarray(inputs['sentence_embeddings'], np.float32)
    sr = np.asarray(inputs['sentence_representation'], np.float32)
    asp = np.asarray(inputs['aspect_embedding'], np.float32)
    mask = np.asarray(inputs['attention_mask'], np.float32)
    common = {
        'ws': np.ascontiguousarray(np.asarray(inputs['ws'], np.float32).astype(np.float16)),
        'wa': np.ascontiguousarray(np.asarray(inputs['wa'], np.float32).astype(np.float16)),
        'wd1': np.ascontiguousarray(np.asarray(inputs['wd1'], np.float32).astype(np.float16)),
        'wd': np.ascontiguousarray(np.asarray(inputs['wd'], np.float32).astype(np.float16)),
        'whs': np.ascontiguousarray(np.asarray(inputs['whs'], np.float32).astype(np.float16)),
        'wihT': np.ascontiguousarray(np.asarray(inputs['w_ih'], np.float32).T.astype(np.float16)),
        'whhT': np.ascontiguousarray(np.asarray(inputs['w_hh'], np.float32).T.astype(np.float16)),
        'w': np.ascontiguousarray(inputs['w'], dtype=np.float32),
    }
    in_maps = []
    for c in range(NCORES):
        sl = slice(c * NB, (c + 1) * NB)
        m = dict(common)
        m['x'] = np.ascontiguousarray(x[sl])
        m['sr'] = np.ascontiguousarray(sr[sl])
        m['asp'] = np.ascontiguousarray(asp[sl])
        m['mask'] = np.ascontiguousarray(mask[sl])
        in_maps.append(m)
    return in_maps



# --------------------------------------------------------------------------
# Harness entry point
# --------------------------------------------------------------------------
B, S_, H_, G_ = 64, 512, 768, 768
NCORES = 8
NB_ = B // NCORES

TRACE = False
TRACE_DIR = None
LAST_EXEC_NS = None

_CACHE = {}


def kernel(**inputs):
    """Full inputs in (as in setup_inputs()), full (64, 768) fp32 output."""
    global LAST_EXEC_NS
    from concourse.bass_utils import run_bass_kernel_spmd
    if 'nc' not in _CACHE:
        _CACHE['nc'] = build_nc(NB_, S_, H_, G_, 3, NCORES)
    in_maps = make_in_maps(inputs, NB_, S_, H_, G_, NCORES)
    kw = {}
    if TRACE:
        kw = dict(trace=True, tmpdir=TRACE_DIR)
    res = run_bass_kernel_spmd(_CACHE['nc'], in_maps, list(range(NCORES)), **kw)
    LAST_EXEC_NS = res.exec_time_ns
    import numpy as _np
    return _np.concatenate([res.results[c]['out'] for c in range(NCORES)],
                           axis=0).astype(_np.float32)

